# revision 5
# baseline (speedup 1.0000x reference)
"""Per-sample 21x21 depthwise conv over (32, 3, 512, 512), 8-way data-parallel
on Trainium2 via Bass/Tile.

Algorithm (W-orientation banded matmul): out[h, w] = sum_{i,j} k[i,j] *
xpad[h+i, w+j] is computed per kernel ROW i as a banded (Toeplitz) matmul
with the HOST-TRANSPOSED padded image as the STATIONARY operand and the
W-direction band of kernel row i as the MOVING operand:

  out[m, n] = sum_p xpadT[wc0+p, h0+i+m] * bandT[p, i, n],
  bandT[p, i, n] = k[i, p-n]   (translation invariant, per-sample tile)

accumulated over the 21 i's in PSUM.  Output tiles are [128 h-rows x nw
w-cols] with h tiled exactly 4x128 and w in chunks nw <= 108 (contraction
window nw+20 <= 128 partitions).  The moving operand streams only nw
columns, so the streamed-element total is 21 x 4 x 512 = 43,008 per image —
20% less than the H-orientation (21 x 5 x 512 = 53,760), because the short
tail chunk pays only its own width (the backend prices matmuls by streamed
moving-operand elements; stationary loads are ~free).  Measured back to
back, W-orientation is ~26% faster than the tuned H-orientation.

Performance structure (empirically tuned against this backend):
- operands bf16 (same PE stream rate as float32r, half DMA/SBUF); image
  transpose + band construction done on host, outside the hot loop.
- ALL input tiles (60 transposed chunk-tiles + 4 weight tiles) DMA'd into
  SBUF once up front; the steady-state body is a gap-free stream of 5040
  matmuls, keeping the PE clock-gate (HAM) at full speed.
- output written as bf16 (PSUM->SBUF copies downcast into a [128, 512]
  collector tile per h-tile; host upcasts to f32), with the 48 output DMAs
  spread round-robin over THREE DMA queues (SP / Pool / ACT) and copies
  alternating DVE / ACT — otherwise the output path backpressures the PSUM
  rotation and the PE drops out of its warm p-state.
- PSUM pool of 6 banks (8 banks measured slower).

Numerics: bf16 operands + bf16 output -> max rel err 3.2e-3 (gate 2e-2).

Sharding: batch 32 -> 4 samples (12 images) per core, no communication.
"""
import sys

sys.path.insert(0, "/opt/trn_rl_repo")

import numpy as np
import ml_dtypes
import concourse.bacc as bacc
import concourse.tile as tile
import concourse.mybir as mybir
from concourse.bass import ds
from concourse.bass_utils import run_bass_kernel_spmd

B, C, H, W = 32, 3, 512, 512
KS, PAD = 21, 10
NCORES = 8
BPC = B // NCORES  # samples per core
T = BPC * C  # images per core
HP = WP = H + 2 * PAD  # 532
MC = 108  # max w_out cols per chunk (contraction window = MC + KS - 1 = 128)
WCHUNKS = [(0, 108), (108, 108), (216, 108), (324, 108), (432, 80)]
HTILES = 4  # 4 x 128 output rows

_nc_cache: dict = {}


def _build_nc(reps: int = 1):
    bf16 = mybir.dt.bfloat16
    f32 = mybir.dt.float32
    nc = bacc.Bacc(
        "TRN2", target_bir_lowering=False, debug=False, enable_asserts=False
    )
    # host-transposed padded image: [T, w, h]
    xT_d = nc.dram_tensor("xT", [T, WP, HP], bf16, kind="ExternalInput")
    # W-band of the per-sample kernel: wb[s, p, i, n] = k[s, i, p - n]
    wb_d = nc.dram_tensor("wb", [BPC, 128, KS, MC], bf16, kind="ExternalInput")
    y_d = nc.dram_tensor("y", [T, H, W], bf16, kind="ExternalOutput")

    with tile.TileContext(nc) as tc:
        with (
            tc.tile_pool(name="wp", bufs=1) as wp,
            tc.tile_pool(name="xp", bufs=1) as xp,
            tc.tile_pool(name="op", bufs=3) as op,
            tc.tile_pool(name="o2", bufs=3) as op2,
            tc.tile_pool(name="ps", bufs=6, space="PSUM") as psp,
        ):
            # hoisted input loads: persistent SBUF tiles for the whole rep loop
            wts = []
            for s in range(BPC):
                wt = wp.tile([128, KS, MC], bf16, tag=f"wt{s}")
                nc.sync.dma_start(
                    wt[:], wb_d[ds(s, 1)].rearrange("o p k m -> (o p) k m")
                )
                wts.append(wt)
            xts = {}
            for t in range(T):
                for wc, (wc0, nw) in enumerate(WCHUNKS):
                    kkw = nw + KS - 1
                    xt = xp.tile([128, HP], bf16, tag=f"xT{t}_{wc}")
                    nc.sync.dma_start(
                        xt[0:kkw, :],
                        xT_d[ds(t, 1), wc0 : wc0 + kkw, :].rearrange(
                            "o p h -> (o p) h"
                        ),
                    )
                    xts[(t, wc)] = xt

            out_qs = [nc.sync.dma_start, nc.gpsimd.dma_start, nc.scalar.dma_start]

            def rep_body():
                idx = 0
                for t in range(T):
                    wt = wts[t // C]
                    for ht in range(HTILES):
                        h0 = ht * 128
                        pool = op if idx % 2 == 0 else op2
                        ot = pool.tile([128, 512], bf16, tag="ot")
                        for wc, (wc0, nw) in enumerate(WCHUNKS):
                            kkw = nw + KS - 1
                            xt = xts[(t, wc)]
                            ps = psp.tile([128, 512], f32, tag="ps")
                            for i in range(KS):
                                nc.tensor.matmul(
                                    ps[0:128, 0:nw],
                                    xt[0:kkw, h0 + i : h0 + i + 128],
                                    wt[0:kkw, i, 0:nw],
                                    start=(i == 0),
                                    stop=(i == KS - 1),
                                )
                            if idx % 2 == 0:
                                nc.vector.tensor_copy(
                                    ot[:, wc0 : wc0 + nw], ps[0:128, 0:nw]
                                )
                            else:
                                nc.scalar.copy(
                                    ot[:, wc0 : wc0 + nw], ps[0:128, 0:nw]
                                )
                        out_qs[idx % 3](
                            y_d[ds(t, 1), h0 : h0 + 128, :].rearrange(
                                "o p w -> (o p) w"
                            ),
                            ot[:],
                        )
                        idx += 1

            if reps == 1:
                rep_body()
            else:
                with tc.For_i(0, reps, 1, hint_engines=(mybir.EngineType.PE,)):
                    rep_body()

    nc.compile()
    return nc


def _host_prep(x: np.ndarray, kern: np.ndarray):
    """Transposed padded image (bf16); per-sample W-banded Toeplitz weights."""
    xpad = np.zeros((B, C, HP, WP), np.float32)
    xpad[:, :, PAD : PAD + H, PAD : PAD + W] = x
    xT = np.ascontiguousarray(xpad.reshape(B * C, HP, WP).transpose(0, 2, 1))

    # band from transposed kernel: Wbs[s, p, i, n] = kT[p-n, i] = k[i, p-n]
    kT = np.ascontiguousarray(kern[:, 0].transpose(0, 2, 1))  # (B, KS, KS)
    Wbs = np.zeros((B, 128, KS, MC), np.float32)
    m = np.arange(MC)
    i = np.arange(KS)
    P = np.broadcast_to(
        i[:, None, None] + m[None, None, :], (KS, KS, MC)
    )  # p = i + m
    J = np.broadcast_to(i[None, :, None], (KS, KS, MC))
    M = np.broadcast_to(m[None, None, :], (KS, KS, MC))
    I = np.broadcast_to(i[:, None, None], (KS, KS, MC))
    Wbs[:, P, J, M] = kT[:, I, J]
    return (
        xT.astype(ml_dtypes.bfloat16),
        Wbs.astype(ml_dtypes.bfloat16),
    )


def _execute(x: np.ndarray, kern: np.ndarray, reps: int = 1) -> np.ndarray:
    if reps not in _nc_cache:
        _nc_cache[reps] = _build_nc(reps)
    nc = _nc_cache[reps]
    xT, Wbs = _host_prep(np.asarray(x), np.asarray(kern))
    in_maps = [
        {
            "xT": np.ascontiguousarray(xT[i * T : (i + 1) * T]),
            "wb": np.ascontiguousarray(Wbs[i * BPC : (i + 1) * BPC]),
        }
        for i in range(NCORES)
    ]
    res = run_bass_kernel_spmd(nc, in_maps, list(range(NCORES)))
    y = np.concatenate(
        [res.results[i]["y"].astype(np.float32) for i in range(NCORES)], axis=0
    )
    return y.reshape(B, C, H, W)


def kernel(x: np.ndarray, kernel: np.ndarray) -> np.ndarray:
    return _execute(x, kernel, reps=1)
